# revision 15
# baseline (speedup 1.0000x reference)
"""Trainium2 Bass kernel for nn_DTFN (mass/flux stencil + vocab decoder).

Strategy (8 NeuronCores, SPMD single NEFF, token-parallel, NO collectives):
  - Sequence-parallel mass+flux: each core computes its S/8=256 positions per
    batch with a K=3 halo; global zero-flux boundaries via per-core edge
    masks that zero the edge flux cells.
  - The flux state lives ONLY as fp8 (e4m3) in mean-shifted units:
      s = 32*(m - C0),  C0 = 0.6875 (exact in e4m3).
    |s| <= ~2 for typical tokens so fp8 resolution on m matches bf16. The
    update s' = max(s + 32*dt*(F[i-1]-F[i]), -32*(C0-EPS)) keeps the
    reference clip-at-EPS semantics exactly.
  - ALL matmuls are fp8 DoubleRow (0.5 cyc/col): mass (emb x w_mass), flux
    h = tanh(lr@w1+b1) and F = softplus(h@w2+b2), and the vocab decode.
    Accuracy is preserved by mean shifts with the rank-1 remainders folded
    into biases on the host (exact, f64):
      lr ~= C0*ones + delta  -> b1' = b1 + C0*colsum(w1)
      h  ~= hbar + dh, hbar = tanh(b1') -> b2' = b2 + hbar@w2
    Biases enter the psum through one f32r rank-1 matmul per 128-chunk
    (bias_row (x) ones), so the scalar-engine activations can run BATCHED
    over all 4 d-chunks (bias of an activation op is per-partition only).
  - Activation tables: softplus = Ln(1+Exp(.)). Exp+Ln share the
    natural_log_exp table; Tanh needs exp_and_others, so each flux step pays
    exactly 2 table swaps (explicitly placed).
  - Token-parallel decoder: each core decodes its OWN 512 tokens against the
    FULL vocab (streams all of w_dec in fp8, no collectives). psum -> int8
    converts (the true wall: ~131k elem/partition across Act+DVE) run as
    [128, 8 vocab-chunks, 256 tok] tiles split greedily across Act and DVE.
  - Output int8 in units of 1/256 logits; host adds the exact rank-1 term
    C0*colsum(w_dec)+b_dec and recomputes the few saturated (boundary)
    tokens from the stored final state s.
"""

import numpy as np
import ml_dtypes
from contextlib import ExitStack

import concourse.bass as bass
import concourse.bacc as bacc
import concourse.mybir as mybir
import concourse.tile as tile
from concourse.bass_utils import run_bass_kernel_spmd
from concourse.bass import _add_dep_helper

F32 = mybir.dt.float32
F32R = mybir.dt.float32r
BF16 = mybir.dt.bfloat16
FP8 = mybir.dt.float8e4
I8 = mybir.dt.int8
I32 = mybir.dt.int32
AF = mybir.ActivationFunctionType
DR = mybir.MatmulPerfMode.DoubleRow
ALU = mybir.AluOpType

V, D, KSTEPS, B, S = 32000, 512, 3, 2, 2048
EPS = 1e-6
NCORES = 8
SC = S // NCORES          # 256 seq positions per batch per core
HALO = KSTEPS             # 3
WM = SC + 2 * HALO        # 262 m-cells per batch per core
WMA = WM + 2              # 264 allocated (pad cells read as 0)
WF = WM - 1               # 261 real flux pairs
WFP = WF + 1              # 262 computed pairs
NTOK = B * WM             # 524 gathered tokens per core
GT = (NTOK + 127) // 128  # 5 gather tiles
DC = D // 128             # 4 d-chunks
C0 = 0.6875               # mean shift, exactly representable in e4m3
C32 = 32.0 * (C0 - EPS)   # state offset: s = 32*mt - C32, mt = m - EPS
SE = 32.0                 # emb scale (host, folded into bf16 emb)
SW = 64.0                 # flux weight scale
SDELT = 32.0              # state scale
SH = 32.0                 # dh scale
SWD = 8.0                 # w_dec scale
KOUT = SDELT * SWD        # 256: psum/int8 units per logit
PSC = 1.0 / 2048.0        # psum -> pre-activation scale (32*64)
VCH = V // 128            # 250 vocab chunks of 128
NWCH = 10                 # w_dec streamed in 10 column chunks
WCOLS = V // NWCH         # 6400 columns per streamed chunk
VPC = WCOLS // 128        # 50 vocab chunks per streamed chunk
CTD = 4                   # vocab chunks per decode psum tile (1 batch)
NCT = (VCH + CTD - 1) // CTD  # 32 decode tiles per batch
GRP = 8                   # vocab chunks per store DMA (both batches)

_CACHE: dict = {}
LAST_RESULTS = None


def _conv_schedule():
    """Greedy least-finish-time assignment of the 2*NCT convert tiles to
    Act/DVE. Costs from the TimelineSim model for [128, n*256] f32->int8."""
    out = []
    load = {"act": 0.0, "dve": 0.0}
    for ct in range(NCT):
        n = min(CTD, VCH - ct * CTD)
        cost = {"act": n * 256 * 0.8333 + 185.0, "dve": n * 256 * 1.0417 + 250.0}
        for _b in range(B):
            e = min(cost, key=lambda k: load[k] + cost[k])
            load[e] += cost[e]
            out.append(e)
    return out


def _build_module():
    nc = bacc.Bacc("TRN2", target_bir_lowering=False, debug=False,
                   num_devices=NCORES)
    from concourse.hw_specs import get_activation_tables
    tables = list(get_activation_tables(nc.m.arch))
    NLE_SET = tables.index("natural_log_exp_and_others")
    E_SET = tables.index("exp_and_others")

    # --- per-core DRAM I/O ---
    t_mskl = nc.dram_tensor("t_mskl", [128, HALO], BF16, kind="ExternalInput")
    t_mskr = nc.dram_tensor("t_mskr", [128, HALO], BF16, kind="ExternalInput")
    t_dt = nc.dram_tensor("t_dt", [128, 1], F32, kind="ExternalInput")
    t_s32 = nc.dram_tensor("t_s32", [128, 1], F32, kind="ExternalInput")
    t_c32 = nc.dram_tensor("t_c32", [128, DC], F32, kind="ExternalInput")
    t_hb = nc.dram_tensor("t_hb", [128, DC], F32, kind="ExternalInput")
    t_wm = nc.dram_tensor("t_wm", [128, DC * D], FP8, kind="ExternalInput")
    t_wf1 = nc.dram_tensor("t_wf1", [128, 2 * DC * D], FP8, kind="ExternalInput")
    t_wf2 = nc.dram_tensor("t_wf2", [128, DC * D], FP8, kind="ExternalInput")
    t_rowm = nc.dram_tensor("t_rowm", [1, D], F32R, kind="ExternalInput")
    t_row1 = nc.dram_tensor("t_row1", [1, D], F32R, kind="ExternalInput")
    t_row2 = nc.dram_tensor("t_row2", [1, D], F32R, kind="ExternalInput")
    t_ones = nc.dram_tensor("t_ones", [1, WFP], F32R, kind="ExternalInput")
    t_eT = nc.dram_tensor("t_eT", [128, DC * GT * 128], FP8, kind="ExternalInput")
    t_wd = nc.dram_tensor("t_wd", [128, DC * V], FP8, kind="ExternalInput")
    t_y = nc.dram_tensor("t_y", [VCH * 128, B * SC], I8, kind="ExternalOutput")
    # final state s = 32*(m - C0), so the host can detect and recompute the
    # saturated (global-boundary) tokens
    t_s = nc.dram_tensor("t_s", [128, B * DC * WMA], FP8, kind="ExternalOutput")

    with tile.TileContext(nc) as tc:
        with ExitStack() as ctx:
            pw = ctx.enter_context(tc.tile_pool(name="pw", bufs=1))
            pwd = ctx.enter_context(tc.tile_pool(name="pwd", bufs=1))
            pfl = ctx.enter_context(tc.tile_pool(name="pfl", bufs=1))

            # ---- small loads ----
            eT = pw.tile([128, DC, GT * 128], FP8)
            nc.sync.dma_start(
                eT[:], t_eT.ap().rearrange("p (g t) -> p g t", g=DC))
            wm_sb = pw.tile([128, DC, D], FP8)
            nc.sync.dma_start(wm_sb[:], t_wm.ap().rearrange("p (g d) -> p g d", g=DC))
            rowm = pw.tile([1, D], F32R)
            nc.sync.dma_start(rowm[:], t_rowm.ap())
            row1 = pw.tile([1, D], F32R)
            nc.sync.dma_start(row1[:], t_row1.ap())
            row2 = pw.tile([1, D], F32R)
            nc.sync.dma_start(row2[:], t_row2.ap())
            ones = pw.tile([1, WFP], F32R)
            nc.sync.dma_start(ones[:], t_ones.ap())
            mskl_sb = pw.tile([128, HALO], BF16)
            nc.sync.dma_start(mskl_sb[:], t_mskl.ap())
            mskr_sb = pw.tile([128, HALO], BF16)
            nc.sync.dma_start(mskr_sb[:], t_mskr.ap())
            dt_sb = pw.tile([128, 1], F32)
            nc.sync.dma_start(dt_sb[:], t_dt.ap())
            s32_sb = pw.tile([128, 1], F32)
            nc.sync.dma_start(s32_sb[:], t_s32.ap())
            c32_sb = pw.tile([128, DC], F32)
            nc.sync.dma_start(c32_sb[:], t_c32.ap())
            hb_sb = pw.tile([128, DC], F32)
            nc.sync.dma_start(hb_sb[:], t_hb.ap())
            wf1_sb = pw.tile([128, 2, DC, D], FP8)
            wf2_sb = pw.tile([128, DC, D], FP8)

            # persistent per-batch state s (fp8) and work tiles
            sT = [pfl.tile([128, DC, WMA], FP8, tag=f"sT{b}", name=f"sT{b}") for b in range(B)]
            hT = [pfl.tile([128, DC, WFP], BF16, tag=f"hT{b}", name=f"hT{b}") for b in range(B)]
            dh8 = [pfl.tile([128, DC, WFP], FP8, tag=f"dh{b}", name=f"dh{b}") for b in range(B)]
            ftmp = [pfl.tile([128, DC, WFP], BF16, tag=f"ft{b}", name=f"ft{b}") for b in range(B)]
            F_sb = [pfl.tile([128, DC, WFP], BF16, tag=f"F{b}", name=f"F{b}") for b in range(B)]
            Fd = [pfl.tile([128, DC, WF - 1], BF16, tag=f"Fd{b}", name=f"Fd{b}") for b in range(B)]
            mt0 = [pfl.tile([128, DC, WM], BF16, tag=f"m0{b}", name=f"m0{b}") for b in range(B)]
            for b in range(B):
                nc.vector.memset(sT[b][:, :, WM:WMA], 0.0)  # pad cells

            # flux weights stream before the wdec chunks
            wf1_i = nc.sync.dma_start(
                wf1_sb[:],
                t_wf1.ap().rearrange("p (s g d) -> p s g d", s=2, g=DC))
            wf2_i = nc.sync.dma_start(
                wf2_sb[:],
                t_wf2.ap().rearrange("p (g d) -> p g d", g=DC))

            # decoder weights: fully resident in SBUF, streamed in slices
            # behind the flux weights so the decode window is free of wdec
            # DMA traffic (stores get the whole DMA device)
            wd_sb = pwd.tile([128, DC, V], FP8, tag="wd")
            t_wd_v = t_wd.ap().rearrange("p (g v) -> p g v", g=DC)
            for c in range(NWCH):
                wi = nc.sync.dma_start(
                    wd_sb[:, :, c * WCOLS:(c + 1) * WCOLS],
                    t_wd_v[:, :, c * WCOLS:(c + 1) * WCOLS])
                if c == 0:
                    _add_dep_helper(wi.ins, wf2_i.ins, sync=True,
                                    reason="wdec stream after flux weights")

            # ---- explicit activation-table management ----
            def load_table(set_id, after=None):
                ld = mybir.InstLoadActFuncSet(
                    name=nc.get_next_instruction_name(), ins=[], outs=[],
                    act_func_set_id=set_id)
                bi = nc.scalar.add_instruction(ld)
                if after is not None:
                    _add_dep_helper(bi.ins, after.ins, sync=True,
                                    reason="table load order")
                return bi

            last_act = [None]

            def anchored_load(set_id):
                ld = load_table(set_id, after=last_act[0])
                last_act[0] = ld
                return ld

            ld_nle = load_table(NLE_SET)
            last_act[0] = ld_nle
            last_mm = [None]

            def warm(dst, n, anchor=None):
                """n tiny chained matmuls into `dst` (a psum slice that the
                next real start=True matmul overwrites) keep the PE p-state
                ramp alive through an engine-chain gap. `anchor` delays the
                chain start so it does not run (and finish) too early."""
                prev = anchor if anchor is not None else last_mm[0]
                for _ in range(n):
                    ji = nc.tensor.matmul(
                        dst[0:64, 0:64], wm_sb[:, 0, 0:64],
                        wm_sb[:, 0, 64:128],
                        start=True, stop=True, skip_group_check=True)
                    if prev is not None:
                        _add_dep_helper(ji.ins, prev.ins, sync=True,
                                        reason="pe warm chain")
                    prev = ji
                last_mm[0] = prev

            # ---- psum pools: A (batch 0), TR (transposes), B (batch 1) ----
            ctxA = ExitStack()
            psA = ctxA.enter_context(tc.tile_pool(name="psA", bufs=1, space="PSUM"))
            # ---- mass (per batch): s0 = 32*(softplus(e@wm + bm) - C8) ----
            first_exp = [None, None]
            last_ln = [None, None]
            last_tanh = [None, None]
            last_fd0 = [None]

            def emit_mass(b, pool):
                pm = pool.tile([128, DC, 512], F32, tag="fx", space="PSUM")
                if b == 0:
                    warm(pm[:, 0, :], 28)
                for q in range(DC):
                    for kk in range(2):
                        nc.tensor.matmul(
                            pm[:, q, 0:WM],
                            wm_sb[:, 2 * kk:2 * kk + 2, q * 128:(q + 1) * 128],
                            eT[:, 2 * kk:2 * kk + 2, b * WM:b * WM + WM],
                            start=(kk == 0), stop=False, perf_mode=DR)
                    bi = nc.tensor.matmul(
                        pm[:, q, 0:WM], rowm[:, q * 128:(q + 1) * 128],
                        ones[:, 0:WM], start=False, stop=True)
                    last_mm[0] = bi
                ei = nc.scalar.activation(ftmp[b][:, :, 0:WM], pm[:, :, 0:WM],
                                          AF.Exp, scale=PSC)
                first_exp[b] = ei
                li = nc.scalar.activation(mt0[b][:], ftmp[b][:, :, 0:WM],
                                          AF.Ln, bias=1.0)
                last_act[0] = li
                last_ln[b] = li
                # s0 = 32*mt - c32 (DVE, fp8 out)
                nc.vector.scalar_tensor_tensor(
                    out=sT[b][:, :, 0:WM], in0=mt0[b][:], scalar=s32_sb[:, 0:1],
                    in1=c32_sb[:, :, None].to_broadcast([128, DC, WM]),
                    op0=ALU.mult, op1=ALU.subtract)

            emit_mass(0, psA)
            _add_dep_helper(first_exp[0].ins, ld_nle.ins, sync=True,
                            reason="mass exp after nle load")
            ctxB = ExitStack()
            psB = ctxB.enter_context(tc.tile_pool(name="psB", bufs=1, space="PSUM"))
            emit_mass(1, psB)
            pools = [psA, psB]

            # ---- K flux steps, batches staggered by half a phase ----
            for k in range(KSTEPS):
                ld_e = anchored_load(E_SET)
                ph = [None, None]
                for b in range(B):
                    ph[b] = pools[b].tile([128, DC, 512], F32, tag="fx",
                                          name=f"ph{b}", space="PSUM")
                    if b == 0:
                        warm(ph[0][:, 0, :], 24,
                             anchor=last_fd0[0] if last_fd0[0] is not None
                             else last_ln[0])
                    for kk in range(2):
                        for q in range(DC):
                            for sh in range(2):
                                nc.tensor.matmul(
                                    ph[b][:, q, 0:WFP],
                                    wf1_sb[:, sh, 2 * kk:2 * kk + 2,
                                           q * 128:(q + 1) * 128],
                                    sT[b][:, 2 * kk:2 * kk + 2, sh:sh + WFP],
                                    start=(sh == 0 and kk == 0), stop=False,
                                    perf_mode=DR)
                    for q in range(DC):
                        bi = nc.tensor.matmul(
                            ph[b][:, q, 0:WFP], row1[:, q * 128:(q + 1) * 128],
                            ones[:, 0:WFP], start=False, stop=True)
                        last_mm[0] = bi
                for b in range(B):
                    ti = nc.scalar.activation(hT[b][:], ph[b][:, :, 0:WFP],
                                              AF.Tanh, scale=PSC)
                    if b == 0:
                        _add_dep_helper(ti.ins, ld_e.ins, sync=True,
                                        reason="tanh after E load")
                    last_act[0] = ti
                    last_tanh[b] = ti
                ld_n = anchored_load(NLE_SET)
                for b in range(B):
                    # dh = 32*h - 32*hbar (fp8)
                    nc.vector.scalar_tensor_tensor(
                        out=dh8[b][:], in0=hT[b][:], scalar=s32_sb[:, 0:1],
                        in1=hb_sb[:, :, None].to_broadcast([128, DC, WFP]),
                        op0=ALU.mult, op1=ALU.subtract)
                pf = [None, None]
                for b in range(B):
                    pf[b] = pools[b].tile([128, DC, 512], F32, tag="fx",
                                          name=f"pf{b}", space="PSUM")
                    if b == 0:
                        warm(pf[0][:, 0, :], 26, anchor=last_tanh[0])
                    for q in range(DC):
                        for kk in range(2):
                            nc.tensor.matmul(
                                pf[b][:, q, 0:WFP],
                                wf2_sb[:, 2 * kk:2 * kk + 2,
                                       q * 128:(q + 1) * 128],
                                dh8[b][:, 2 * kk:2 * kk + 2, 0:WFP],
                                start=(kk == 0), stop=False, perf_mode=DR)
                        bi = nc.tensor.matmul(
                            pf[b][:, q, 0:WFP], row2[:, q * 128:(q + 1) * 128],
                            ones[:, 0:WFP], start=False, stop=True)
                        last_mm[0] = bi
                # exp runs in the E table (exp_and_others has Exp); the NLE
                # load slots between the exps and the lns
                for b in range(B):
                    ei = nc.scalar.activation(ftmp[b][:], pf[b][:, :, 0:WFP],
                                              AF.Exp, scale=PSC)
                    if b == 0:
                        _add_dep_helper(ei.ins, ld_n.ins, sync=True,
                                        reason="exp after nle load")
                    last_act[0] = ei
                    nc.gpsimd.tensor_tensor(
                        out=ftmp[b][:, :, 0:HALO], in0=ftmp[b][:, :, 0:HALO],
                        in1=mskl_sb[:, None, :].to_broadcast([128, DC, HALO]),
                        op=ALU.mult)
                    nc.gpsimd.tensor_tensor(
                        out=ftmp[b][:, :, WF - HALO:WF],
                        in0=ftmp[b][:, :, WF - HALO:WF],
                        in1=mskr_sb[:, None, :].to_broadcast([128, DC, HALO]),
                        op=ALU.mult)
                    li = nc.scalar.activation(F_sb[b][:], ftmp[b][:], AF.Ln,
                                              bias=1.0)
                    last_act[0] = li
                    last_ln[b] = li
                # s' = 32*dt*(F[i-1]-F[i]) + s.  The clip-at-EPS max is
                # dropped: it only binds at the global boundary cells, which
                # the host recomputes exactly.  Batch 0 (the chain batch)
                # updates in chunk-pair halves so the next h matmuls (kk
                # outer) can start after the first half.
                for b in range(B):
                    halves = [(0, 2), (2, 4)] if b == 0 else [(0, 4)]
                    for (q0, q1) in halves:
                        qn = q1 - q0
                        fi = nc.vector.tensor_tensor(
                            out=Fd[b][:, q0:q1, :],
                            in0=F_sb[b][:, q0:q1, 0:WF - 1],
                            in1=F_sb[b][:, q0:q1, 1:WF], op=ALU.subtract)
                        if b == 0 and q0 == 0:
                            last_fd0[0] = fi
                        nc.vector.scalar_tensor_tensor(
                            out=sT[b][:, q0:q1, 1:WM - 1],
                            in0=Fd[b][:, q0:q1, :],
                            scalar=dt_sb[:, 0:1],
                            in1=sT[b][:, q0:q1, 1:WM - 1],
                            op0=ALU.mult, op1=ALU.add)

            # final state out (host overflow detection + recompute)
            t_s_v = t_s.ap().rearrange("p (b g t) -> p b g t", b=B, g=DC)
            for b in range(B):
                nc.sync.dma_start(t_s_v[:, b, :, :], sT[b][:])

            ctxB.close()
            ctxA.close()

            # ---- decode: fp8 DR matmuls + psum->int8 converts + stores ----
            sched = _conv_schedule()
            with ExitStack() as ctxd:
                psd = ctxd.enter_context(
                    tc.tile_pool(name="psd", bufs=4, space="PSUM"))
                po = ctxd.enter_context(tc.tile_pool(name="po", bufs=3))
                osb = None
                si = 0
                for ct in range(NCT):
                    n = min(CTD, VCH - ct * CTD)
                    g = ct % 2  # position within the store group
                    if g == 0:
                        osb = po.tile([128, GRP, 512], I8, tag="osb")
                    for b in range(B):
                        pd = psd.tile([128, CTD, 256], F32, tag="pd",
                                      space="PSUM")
                        if ct == 0 and b == 0:
                            warm(pd[:, 0, :], 20)
                        for j in range(n):
                            v = ct * CTD + j
                            for kk in range(2):
                                nc.tensor.matmul(
                                    pd[:, j, :],
                                    wd_sb[:, 2 * kk:2 * kk + 2,
                                          v * 128:(v + 1) * 128],
                                    sT[b][:, 2 * kk:2 * kk + 2,
                                          HALO:HALO + SC],
                                    start=(kk == 0), stop=(kk == 1),
                                    perf_mode=DR)
                        dst = osb[:, g * CTD:g * CTD + n, b * SC:(b + 1) * SC]
                        if sched[si] == "act":
                            nc.scalar.activation(dst, pd[:, 0:n, :], AF.Copy)
                        else:
                            nc.vector.tensor_copy(dst, pd[:, 0:n, :])
                        si += 1
                    if g == 1 or ct == NCT - 1:
                        r0 = (ct - g) * CTD * 128
                        r1 = (ct * CTD + n) * 128
                        dst = t_y.ap()[r0:r1, :]
                        nc.gpsimd.dma_start(
                            dst.rearrange("(j p) t -> p j t", p=128),
                            osb[:, 0:g * CTD + n, :])

    nc.compile()
    return nc


def _get_module(variant="all"):
    key = f"nc:{variant}"
    if key not in _CACHE:
        _CACHE[key] = _build_module()
    return _CACHE[key]


def _prep_inputs(x, emb, w_mass, b_mass, w_f1, b_f1, w_f2, b_f2, cfl_raw,
                 w_dec, b_dec):
    x = np.asarray(x)
    emb = np.asarray(emb, dtype=np.float64)
    w_mass = np.asarray(w_mass, dtype=np.float64)
    b_mass = np.asarray(b_mass, dtype=np.float64)
    w_f1 = np.asarray(w_f1, dtype=np.float64)
    b_f1 = np.asarray(b_f1, dtype=np.float64)
    w_f2 = np.asarray(w_f2, dtype=np.float64)
    b_f2 = np.asarray(b_f2, dtype=np.float64)
    w_dec = np.asarray(w_dec, dtype=np.float32)
    dt = float(1.0 / (1.0 + np.exp(-np.float64(np.asarray(cfl_raw)))))

    bf16 = ml_dtypes.bfloat16
    fp8 = ml_dtypes.float8_e4m3

    # host-side embedding gather + transpose + fp8 quantization (pure data
    # movement; also shrinks per-core input traffic vs shipping all of emb)
    emb8 = (np.asarray(emb, np.float64) * SE).astype(bf16).astype(fp8)
    wm_in = np.ascontiguousarray(
        (w_mass * SW).reshape(DC, 128, D).transpose(1, 0, 2)
        .reshape(128, DC * D).astype(fp8))
    wf1_in = np.ascontiguousarray(
        (w_f1 * SW).reshape(2, DC, 128, D).transpose(2, 0, 1, 3)
        .reshape(128, 2 * DC * D).astype(fp8))
    wf2_in = np.ascontiguousarray(
        (w_f2 * SW).reshape(DC, 128, D).transpose(1, 0, 2)
        .reshape(128, DC * D).astype(fp8))
    wd_in = np.ascontiguousarray(
        (np.asarray(w_dec, np.float64) * SWD).reshape(DC, 128, V)
        .transpose(1, 0, 2).reshape(128, DC * V).astype(fp8))

    # host-exact bias folds (f64)
    b1p = b_f1 + C0 * w_f1.sum(0)                  # [D]
    hbar = np.tanh(b1p)                            # [D]
    b2p = b_f2 + hbar @ w_f2                       # [D]
    rowm = np.ascontiguousarray((2048.0 * b_mass)[None, :].astype(np.float32))
    row1 = np.ascontiguousarray((2048.0 * b1p)[None, :].astype(np.float32))
    row2 = np.ascontiguousarray((2048.0 * b2p)[None, :].astype(np.float32))
    ones_in = np.ones((1, WFP), np.float32)
    hb_in = np.ascontiguousarray(
        (SH * hbar).reshape(DC, 128).T.astype(np.float32))
    c32_in = np.full((128, DC), C32, np.float32)
    s32_in = np.full((128, 1), 32.0, np.float32)
    dt_in = np.full((128, 1), 32.0 * dt, dtype=np.float32)

    in_maps = []
    for c in range(NCORES):
        sedge = c * SC - HALO
        idx = np.zeros(GT * 128, dtype=np.int32)
        for b in range(B):
            t = np.arange(WM)
            sc = np.clip(sedge + t, 0, S - 1)
            idx[b * WM:(b + 1) * WM] = x[b, sc]
        # eT[p, g, slot] = emb8[idx[slot], g*128 + p]
        eg = emb8[idx]                                # [GT*128, D]
        eT_in = np.ascontiguousarray(
            eg.T.reshape(DC, 128, GT * 128).transpose(1, 0, 2)
            .reshape(128, DC * GT * 128))

        j = np.arange(WFP)
        gp = sedge + j
        fm = ((gp >= 0) & (gp <= S - 2)).astype(np.float32)
        mskl = np.ascontiguousarray(
            np.broadcast_to(fm[0:HALO], (128, HALO)).astype(bf16))
        mskr = np.ascontiguousarray(
            np.broadcast_to(fm[WF - HALO:WF], (128, HALO)).astype(bf16))

        in_maps.append({
            "t_eT": eT_in, "t_mskl": mskl, "t_mskr": mskr, "t_dt": dt_in,
            "t_s32": s32_in, "t_c32": c32_in, "t_hb": hb_in,
            "t_wm": wm_in, "t_wf1": wf1_in, "t_wf2": wf2_in,
            "t_rowm": rowm, "t_row1": row1, "t_row2": row2,
            "t_ones": ones_in, "t_wd": wd_in,
        })
    return in_maps


def _edge_exact(inputs, y, edge=8, pad=4):
    """Exact f64 reference for the first/last `edge` sequence positions.
    The device drops the clip-at-EPS and quantizes the large boundary
    deltas in fp8; both effects are confined to cells within K=3 of the
    global edges, so recompute those tokens' logits on the host."""
    x = np.asarray(inputs["x"])
    emb = np.asarray(inputs["emb"], np.float64)
    w_mass = np.asarray(inputs["w_mass"], np.float64)
    b_mass = np.asarray(inputs["b_mass"], np.float64)
    w_f1 = np.asarray(inputs["w_f1"], np.float64)
    b_f1 = np.asarray(inputs["b_f1"], np.float64)
    w_f2 = np.asarray(inputs["w_f2"], np.float64)
    b_f2 = np.asarray(inputs["b_f2"], np.float64)
    w_dec = np.asarray(inputs["w_dec"], np.float64)
    b_dec = np.asarray(inputs["b_dec"], np.float64)
    dt = 1.0 / (1.0 + np.exp(-np.float64(np.asarray(inputs["cfl_raw"]))))

    def sp(z):
        return np.logaddexp(0.0, z)

    W = edge + pad
    for side in (0, 1):
        pos = np.arange(0, W) if side == 0 else np.arange(S - W, S)
        m = sp(emb[x[:, pos]] @ w_mass + b_mass) + EPS
        for _ in range(KSTEPS):
            lr = np.concatenate([m[:, :-1], m[:, 1:]], axis=-1)
            F = sp(np.tanh(lr @ w_f1 + b_f1) @ w_f2 + b_f2)
            zpad = np.zeros((B, 1, D))
            if side == 0:
                Fl = np.concatenate([zpad, F], axis=1)       # global edge
                Fr = np.concatenate([F, F[:, -1:]], axis=1)  # window edge
            else:
                Fl = np.concatenate([F[:, 0:1], F], axis=1)
                Fr = np.concatenate([F, zpad], axis=1)
            m = np.clip(m + dt * (Fl - Fr), EPS, None)
        logits = m @ w_dec + b_dec
        if side == 0:
            y[:, 0:edge, :] = logits[:, 0:edge, :].astype(np.float32)
        else:
            y[:, S - edge:S, :] = logits[:, pad:, :].astype(np.float32)


def kernel(**inputs) -> np.ndarray:
    global LAST_RESULTS
    import os
    nc = _get_module()
    in_maps = _prep_inputs(**inputs)
    try:
        res = run_bass_kernel_spmd(nc, in_maps, core_ids=list(range(NCORES)))
    except (ImportError, ModuleNotFoundError):
        if os.environ.get("BASS_TRACE"):
            os.environ["BASS_NEVER_TRACE"] = "1"
            res = run_bass_kernel_spmd(nc, in_maps,
                                       core_ids=list(range(NCORES)))
        else:
            raise
    LAST_RESULTS = res

    w_dec = np.asarray(inputs["w_dec"], dtype=np.float32)
    b_dec = np.asarray(inputs["b_dec"], dtype=np.float32)
    L0 = (C0 * w_dec.sum(0) + b_dec).astype(np.float32)  # [V]
    wmax2 = float(np.linalg.norm(w_dec, axis=0).max())
    lim = (127.0 / KOUT) / 1.08

    y = np.empty((B, S, V), dtype=np.float32)
    for c in range(NCORES):
        blk = res.results[c]["t_y"].astype(np.float32) / KOUT  # [V, B*SC]
        blk = blk.reshape(V, B, SC).transpose(1, 2, 0)         # [B, SC, V]
        y[:, c * SC:(c + 1) * SC, :] = blk + L0[None, None, :]
        # final state: [128, B, DC, WMA]; delta[t, d] = s[p, b, g, HALO+t]/32
        ss = res.results[c]["t_s"].astype(np.float32).reshape(
            128, B, DC, WMA)
        for b in range(B):
            sd = ss[:, b, :, HALO:HALO + SC]            # [128, DC, SC]
            delta = sd.transpose(2, 1, 0).reshape(SC, D) / SDELT
            bt = np.linalg.norm(delta, axis=1)
            for t in np.nonzero(bt * wmax2 > lim)[0]:
                s_pos = c * SC + t
                y[b, s_pos, :] = (delta[t] @ w_dec) + L0
    _edge_exact(inputs, y)
    return y


# revision 17
# speedup vs baseline: 1.0915x; 1.0915x over previous
"""Trainium2 Bass kernel for nn_DTFN (mass/flux stencil + vocab decoder).

Strategy (8 NeuronCores, SPMD single NEFF, token-parallel, NO collectives):
  - Sequence-parallel mass+flux: each core computes its S/8=256 positions per
    batch with a K=3 halo; global zero-flux boundaries via per-core edge
    masks that zero the edge flux cells.
  - The flux state lives ONLY as fp8 (e4m3) in mean-shifted units:
      s = 32*(m - C0),  C0 = 0.6875 (exact in e4m3).
    |s| <= ~2 for typical tokens so fp8 resolution on m matches bf16. The
    update s' = max(s + 32*dt*(F[i-1]-F[i]), -32*(C0-EPS)) keeps the
    reference clip-at-EPS semantics exactly.
  - ALL matmuls are fp8 DoubleRow (0.5 cyc/col): mass (emb x w_mass), flux
    h = tanh(lr@w1+b1) and F = softplus(h@w2+b2), and the vocab decode.
    Accuracy is preserved by mean shifts with the rank-1 remainders folded
    into biases on the host (exact, f64):
      lr ~= C0*ones + delta  -> b1' = b1 + C0*colsum(w1)
      h  ~= hbar + dh, hbar = tanh(b1') -> b2' = b2 + hbar@w2
    Biases enter the psum through one f32r rank-1 matmul per 128-chunk
    (bias_row (x) ones), so the scalar-engine activations can run BATCHED
    over all 4 d-chunks (bias of an activation op is per-partition only).
  - Activation tables: softplus = Ln(1+Exp(.)). Exp+Ln share the
    natural_log_exp table; Tanh needs exp_and_others, so each flux step pays
    exactly 2 table swaps (explicitly placed).
  - Token-parallel decoder: each core decodes its OWN 512 tokens against the
    FULL vocab (streams all of w_dec in fp8, no collectives). psum -> int8
    converts (the true wall: ~131k elem/partition across Act+DVE) run as
    [128, 8 vocab-chunks, 256 tok] tiles split greedily across Act and DVE.
  - Output int8 in units of 1/256 logits; host adds the exact rank-1 term
    C0*colsum(w_dec)+b_dec and recomputes the few saturated (boundary)
    tokens from the stored final state s.
"""

import numpy as np
import ml_dtypes
from contextlib import ExitStack

import concourse.bass as bass
import concourse.bacc as bacc
import concourse.mybir as mybir
import concourse.tile as tile
from concourse.bass_utils import run_bass_kernel_spmd
from concourse.bass import _add_dep_helper

F32 = mybir.dt.float32
F32R = mybir.dt.float32r
BF16 = mybir.dt.bfloat16
FP8 = mybir.dt.float8e4
I8 = mybir.dt.int8
I32 = mybir.dt.int32
AF = mybir.ActivationFunctionType
DR = mybir.MatmulPerfMode.DoubleRow
ALU = mybir.AluOpType

V, D, KSTEPS, B, S = 32000, 512, 3, 2, 2048
EPS = 1e-6
NCORES = 8
SC = S // NCORES          # 256 seq positions per batch per core
HALO = KSTEPS             # 3
WM = SC + 2 * HALO        # 262 m-cells per batch per core
WMA = WM + 2              # 264 allocated (pad cells read as 0)
WF = WM - 1               # 261 real flux pairs
WFP = WF + 1              # 262 computed pairs
NTOK = B * WM             # 524 gathered tokens per core
GT = (NTOK + 127) // 128  # 5 gather tiles
DC = D // 128             # 4 d-chunks
C0 = 0.6875               # mean shift, exactly representable in e4m3
C32 = 32.0 * (C0 - EPS)   # state offset: s = 32*mt - C32, mt = m - EPS
SE = 32.0                 # emb scale (host, folded into bf16 emb)
SW = 64.0                 # flux weight scale
SDELT = 32.0              # state scale
SH = 32.0                 # dh scale
SWD = 8.0                 # w_dec scale
KOUT = SDELT * SWD        # 256: psum/int8 units per logit
PSC = 1.0 / 2048.0        # psum -> pre-activation scale (32*64)
VCH = V // 128            # 250 vocab chunks of 128
NWCH = 10                 # w_dec streamed in 10 column chunks
WCOLS = V // NWCH         # 6400 columns per streamed chunk
VPC = WCOLS // 128        # 50 vocab chunks per streamed chunk
CTD = 4                   # vocab chunks per decode psum tile (1 batch)
NCT = (VCH + CTD - 1) // CTD  # 32 decode tiles per batch
GRP = 16                  # vocab chunks per store DMA (both batches)

_CACHE: dict = {}
LAST_RESULTS = None


def _conv_schedule():
    """Greedy least-finish-time assignment of the 2*NCT convert tiles to
    Act/DVE. Costs from the TimelineSim model for [128, n*256] f32->int8."""
    out = []
    load = {"act": 0.0, "dve": 0.0}
    for ct in range(NCT):
        n = min(CTD, VCH - ct * CTD)
        cost = {"act": n * 256 * 0.8333 + 185.0, "dve": n * 256 * 1.0417 + 250.0}
        for _b in range(B):
            e = min(cost, key=lambda k: load[k] + cost[k])
            load[e] += cost[e]
            out.append(e)
    return out


def _build_module():
    nc = bacc.Bacc("TRN2", target_bir_lowering=False, debug=False,
                   num_devices=NCORES)
    from concourse.hw_specs import get_activation_tables
    tables = list(get_activation_tables(nc.m.arch))
    NLE_SET = tables.index("natural_log_exp_and_others")
    E_SET = tables.index("exp_and_others")

    # --- per-core DRAM I/O ---
    t_mskl = nc.dram_tensor("t_mskl", [128, HALO], BF16, kind="ExternalInput")
    t_mskr = nc.dram_tensor("t_mskr", [128, HALO], BF16, kind="ExternalInput")
    t_dt = nc.dram_tensor("t_dt", [128, 1], F32, kind="ExternalInput")
    t_s32 = nc.dram_tensor("t_s32", [128, 1], F32, kind="ExternalInput")
    t_c32 = nc.dram_tensor("t_c32", [128, DC], F32, kind="ExternalInput")
    t_hb = nc.dram_tensor("t_hb", [128, DC], F32, kind="ExternalInput")
    t_wm = nc.dram_tensor("t_wm", [128, DC * D], FP8, kind="ExternalInput")
    t_wf1 = nc.dram_tensor("t_wf1", [128, 2 * DC * D], FP8, kind="ExternalInput")
    t_wf2 = nc.dram_tensor("t_wf2", [128, DC * D], FP8, kind="ExternalInput")
    t_rowm = nc.dram_tensor("t_rowm", [1, D], F32R, kind="ExternalInput")
    t_row1 = nc.dram_tensor("t_row1", [1, D], F32R, kind="ExternalInput")
    t_row2 = nc.dram_tensor("t_row2", [1, D], F32R, kind="ExternalInput")
    t_ones = nc.dram_tensor("t_ones", [1, WFP], F32R, kind="ExternalInput")
    t_eT = nc.dram_tensor("t_eT", [128, DC * GT * 128], FP8, kind="ExternalInput")
    t_wd = nc.dram_tensor("t_wd", [128, DC * V], FP8, kind="ExternalInput")
    t_y = nc.dram_tensor("t_y", [VCH * 128, B * SC], I8, kind="ExternalOutput")
    # final state s = 32*(m - C0), so the host can detect and recompute the
    # saturated (global-boundary) tokens
    t_s = nc.dram_tensor("t_s", [128, B * DC * WMA], FP8, kind="ExternalOutput")

    with tile.TileContext(nc) as tc:
        with ExitStack() as ctx:
            pw = ctx.enter_context(tc.tile_pool(name="pw", bufs=1))
            pwd = ctx.enter_context(tc.tile_pool(name="pwd", bufs=1))
            pfl = ctx.enter_context(tc.tile_pool(name="pfl", bufs=1))

            # ---- small loads ----
            eT = pw.tile([128, DC, GT * 128], FP8)
            nc.sync.dma_start(
                eT[:], t_eT.ap().rearrange("p (g t) -> p g t", g=DC))
            wm_sb = pw.tile([128, DC, D], FP8)
            nc.sync.dma_start(wm_sb[:], t_wm.ap().rearrange("p (g d) -> p g d", g=DC))
            rowm = pw.tile([1, D], F32R)
            nc.sync.dma_start(rowm[:], t_rowm.ap())
            row1 = pw.tile([1, D], F32R)
            nc.sync.dma_start(row1[:], t_row1.ap())
            row2 = pw.tile([1, D], F32R)
            nc.sync.dma_start(row2[:], t_row2.ap())
            ones = pw.tile([1, WFP], F32R)
            nc.sync.dma_start(ones[:], t_ones.ap())
            mskl_sb = pw.tile([128, HALO], BF16)
            nc.sync.dma_start(mskl_sb[:], t_mskl.ap())
            mskr_sb = pw.tile([128, HALO], BF16)
            nc.sync.dma_start(mskr_sb[:], t_mskr.ap())
            dt_sb = pw.tile([128, 1], F32)
            nc.sync.dma_start(dt_sb[:], t_dt.ap())
            s32_sb = pw.tile([128, 1], F32)
            nc.sync.dma_start(s32_sb[:], t_s32.ap())
            c32_sb = pw.tile([128, DC], F32)
            nc.sync.dma_start(c32_sb[:], t_c32.ap())
            hb_sb = pw.tile([128, DC], F32)
            nc.sync.dma_start(hb_sb[:], t_hb.ap())
            wf1_sb = pw.tile([128, 2, DC, D], FP8)
            wf2_sb = pw.tile([128, DC, D], FP8)

            # persistent per-batch state s (fp8) and work tiles
            sT = [pfl.tile([128, DC, WMA], FP8, tag=f"sT{b}", name=f"sT{b}") for b in range(B)]
            hT = [pfl.tile([128, DC, WFP], BF16, tag=f"hT{b}", name=f"hT{b}") for b in range(B)]
            dh8 = [pfl.tile([128, DC, WFP], FP8, tag=f"dh{b}", name=f"dh{b}") for b in range(B)]
            ftmp = [pfl.tile([128, DC, WFP], BF16, tag=f"ft{b}", name=f"ft{b}") for b in range(B)]
            F_sb = [pfl.tile([128, DC, WFP], BF16, tag=f"F{b}", name=f"F{b}") for b in range(B)]
            Fd = [pfl.tile([128, DC, WF - 1], BF16, tag=f"Fd{b}", name=f"Fd{b}") for b in range(B)]
            mt0 = [pfl.tile([128, DC, WM], BF16, tag=f"m0{b}", name=f"m0{b}") for b in range(B)]
            for b in range(B):
                nc.vector.memset(sT[b][:, :, WM:WMA], 0.0)  # pad cells

            # flux weights stream before the wdec chunks
            wf1_i = nc.sync.dma_start(
                wf1_sb[:],
                t_wf1.ap().rearrange("p (s g d) -> p s g d", s=2, g=DC))
            wf2_i = nc.sync.dma_start(
                wf2_sb[:],
                t_wf2.ap().rearrange("p (g d) -> p g d", g=DC))

            # decoder weights: fully resident in SBUF, streamed in slices
            # behind the flux weights so the decode window is free of wdec
            # DMA traffic (stores get the whole DMA device)
            wd_sb = pwd.tile([128, DC, V], FP8, tag="wd")
            t_wd_v = t_wd.ap().rearrange("p (g v) -> p g v", g=DC)
            for c in range(NWCH):
                wi = nc.sync.dma_start(
                    wd_sb[:, :, c * WCOLS:(c + 1) * WCOLS],
                    t_wd_v[:, :, c * WCOLS:(c + 1) * WCOLS])
                if c == 0:
                    _add_dep_helper(wi.ins, wf2_i.ins, sync=True,
                                    reason="wdec stream after flux weights")

            # ---- explicit activation-table management ----
            def load_table(set_id, after=None):
                ld = mybir.InstLoadActFuncSet(
                    name=nc.get_next_instruction_name(), ins=[], outs=[],
                    act_func_set_id=set_id)
                bi = nc.scalar.add_instruction(ld)
                if after is not None:
                    _add_dep_helper(bi.ins, after.ins, sync=True,
                                    reason="table load order")
                return bi

            last_act = [None]

            def anchored_load(set_id):
                ld = load_table(set_id, after=last_act[0])
                last_act[0] = ld
                return ld

            ld_nle = load_table(NLE_SET)
            last_act[0] = ld_nle
            last_mm = [None]

            def warm(dst, n, anchor=None):
                """n tiny chained matmuls into `dst` (a psum slice that the
                next real start=True matmul overwrites) keep the PE p-state
                ramp alive through an engine-chain gap. `anchor` delays the
                chain start so it does not run (and finish) too early."""
                prev = anchor if anchor is not None else last_mm[0]
                for _ in range(n):
                    ji = nc.tensor.matmul(
                        dst[0:64, 0:64], wm_sb[:, 0, 0:64],
                        wm_sb[:, 0, 64:128],
                        start=True, stop=True, skip_group_check=True)
                    if prev is not None:
                        _add_dep_helper(ji.ins, prev.ins, sync=True,
                                        reason="pe warm chain")
                    prev = ji
                last_mm[0] = prev

            # ---- psum pools: A (batch 0), TR (transposes), B (batch 1) ----
            ctxA = ExitStack()
            psA = ctxA.enter_context(tc.tile_pool(name="psA", bufs=1, space="PSUM"))
            # ---- mass (per batch): s0 = 32*(softplus(e@wm + bm) - C8) ----
            first_exp = [None, None]
            last_ln = [None, None]
            last_tanh = [None, None]
            last_fd0 = [None]

            def emit_mass(b, pool):
                pm = pool.tile([128, DC, 512], F32, tag="fx", space="PSUM")
                if b == 0:
                    warm(pm[:, 0, :], 14)
                for q in range(DC):
                    for kk in range(2):
                        nc.tensor.matmul(
                            pm[:, q, 0:WM],
                            wm_sb[:, 2 * kk:2 * kk + 2, q * 128:(q + 1) * 128],
                            eT[:, 2 * kk:2 * kk + 2, b * WM:b * WM + WM],
                            start=(kk == 0), stop=False, perf_mode=DR)
                    bi = nc.tensor.matmul(
                        pm[:, q, 0:WM], rowm[:, q * 128:(q + 1) * 128],
                        ones[:, 0:WM], start=False, stop=True)
                    last_mm[0] = bi
                ei = nc.scalar.activation(ftmp[b][:, :, 0:WM], pm[:, :, 0:WM],
                                          AF.Exp, scale=PSC)
                first_exp[b] = ei
                li = nc.scalar.activation(mt0[b][:], ftmp[b][:, :, 0:WM],
                                          AF.Ln, bias=1.0)
                last_act[0] = li
                last_ln[b] = li
                # s0 = 32*mt - c32 (DVE, fp8 out)
                nc.vector.scalar_tensor_tensor(
                    out=sT[b][:, :, 0:WM], in0=mt0[b][:], scalar=s32_sb[:, 0:1],
                    in1=c32_sb[:, :, None].to_broadcast([128, DC, WM]),
                    op0=ALU.mult, op1=ALU.subtract)

            emit_mass(0, psA)
            _add_dep_helper(first_exp[0].ins, ld_nle.ins, sync=True,
                            reason="mass exp after nle load")
            ctxB = ExitStack()
            psB = ctxB.enter_context(tc.tile_pool(name="psB", bufs=1, space="PSUM"))
            emit_mass(1, psB)
            pools = [psA, psB]

            # ---- K flux steps, batches staggered by half a phase ----
            for k in range(KSTEPS):
                ld_e = anchored_load(E_SET)
                ph = [None, None]
                for b in range(B):
                    ph[b] = pools[b].tile([128, DC, 512], F32, tag="fx",
                                          name=f"ph{b}", space="PSUM")
                    if b == 0:
                        warm(ph[0][:, 0, :], 24,
                             anchor=last_fd0[0] if last_fd0[0] is not None
                             else last_ln[0])
                    for kk in range(2):
                        for q in range(DC):
                            for sh in range(2):
                                nc.tensor.matmul(
                                    ph[b][:, q, 0:WFP],
                                    wf1_sb[:, sh, 2 * kk:2 * kk + 2,
                                           q * 128:(q + 1) * 128],
                                    sT[b][:, 2 * kk:2 * kk + 2, sh:sh + WFP],
                                    start=(sh == 0 and kk == 0), stop=False,
                                    perf_mode=DR)
                    for q in range(DC):
                        bi = nc.tensor.matmul(
                            ph[b][:, q, 0:WFP], row1[:, q * 128:(q + 1) * 128],
                            ones[:, 0:WFP], start=False, stop=True)
                        last_mm[0] = bi
                for b in range(B):
                    ti = nc.scalar.activation(hT[b][:], ph[b][:, :, 0:WFP],
                                              AF.Tanh, scale=PSC)
                    if b == 0:
                        _add_dep_helper(ti.ins, ld_e.ins, sync=True,
                                        reason="tanh after E load")
                    last_act[0] = ti
                    last_tanh[b] = ti
                ld_n = anchored_load(NLE_SET)
                for b in range(B):
                    # dh = 32*h - 32*hbar (fp8)
                    nc.vector.scalar_tensor_tensor(
                        out=dh8[b][:], in0=hT[b][:], scalar=s32_sb[:, 0:1],
                        in1=hb_sb[:, :, None].to_broadcast([128, DC, WFP]),
                        op0=ALU.mult, op1=ALU.subtract)
                pf = [None, None]
                for b in range(B):
                    pf[b] = pools[b].tile([128, DC, 512], F32, tag="fx",
                                          name=f"pf{b}", space="PSUM")
                    if b == 0:
                        warm(pf[0][:, 0, :], 26, anchor=last_tanh[0])
                    for q in range(DC):
                        for kk in range(2):
                            nc.tensor.matmul(
                                pf[b][:, q, 0:WFP],
                                wf2_sb[:, 2 * kk:2 * kk + 2,
                                       q * 128:(q + 1) * 128],
                                dh8[b][:, 2 * kk:2 * kk + 2, 0:WFP],
                                start=(kk == 0), stop=False, perf_mode=DR)
                        bi = nc.tensor.matmul(
                            pf[b][:, q, 0:WFP], row2[:, q * 128:(q + 1) * 128],
                            ones[:, 0:WFP], start=False, stop=True)
                        last_mm[0] = bi
                # exp runs in the E table (exp_and_others has Exp); the NLE
                # load slots between the exps and the lns
                for b in range(B):
                    ei = nc.scalar.activation(ftmp[b][:], pf[b][:, :, 0:WFP],
                                              AF.Exp, scale=PSC)
                    if b == 0:
                        _add_dep_helper(ei.ins, ld_n.ins, sync=True,
                                        reason="exp after nle load")
                    last_act[0] = ei
                    nc.gpsimd.tensor_tensor(
                        out=ftmp[b][:, :, 0:HALO], in0=ftmp[b][:, :, 0:HALO],
                        in1=mskl_sb[:, None, :].to_broadcast([128, DC, HALO]),
                        op=ALU.mult)
                    nc.gpsimd.tensor_tensor(
                        out=ftmp[b][:, :, WF - HALO:WF],
                        in0=ftmp[b][:, :, WF - HALO:WF],
                        in1=mskr_sb[:, None, :].to_broadcast([128, DC, HALO]),
                        op=ALU.mult)
                    li = nc.scalar.activation(F_sb[b][:], ftmp[b][:], AF.Ln,
                                              bias=1.0)
                    last_act[0] = li
                    last_ln[b] = li
                # s' = 32*dt*(F[i-1]-F[i]) + s.  The clip-at-EPS max is
                # dropped: it only binds at the global boundary cells, which
                # the host recomputes exactly.  Batch 0 (the chain batch)
                # updates in chunk-pair halves so the next h matmuls (kk
                # outer) can start after the first half.
                for b in range(B):
                    halves = [(0, 2), (2, 4)] if b == 0 else [(0, 4)]
                    for (q0, q1) in halves:
                        qn = q1 - q0
                        fi = nc.vector.tensor_tensor(
                            out=Fd[b][:, q0:q1, :],
                            in0=F_sb[b][:, q0:q1, 0:WF - 1],
                            in1=F_sb[b][:, q0:q1, 1:WF], op=ALU.subtract)
                        if b == 0 and q0 == 0:
                            last_fd0[0] = fi
                        nc.vector.scalar_tensor_tensor(
                            out=sT[b][:, q0:q1, 1:WM - 1],
                            in0=Fd[b][:, q0:q1, :],
                            scalar=dt_sb[:, 0:1],
                            in1=sT[b][:, q0:q1, 1:WM - 1],
                            op0=ALU.mult, op1=ALU.add)

            # final state out (host overflow detection + recompute)
            t_s_v = t_s.ap().rearrange("p (b g t) -> p b g t", b=B, g=DC)
            for b in range(B):
                nc.sync.dma_start(t_s_v[:, b, :, :], sT[b][:])

            ctxB.close()
            ctxA.close()

            # ---- decode: fp8 DR matmuls + psum->int8 converts + stores ----
            sched = _conv_schedule()
            with ExitStack() as ctxd:
                psd = ctxd.enter_context(
                    tc.tile_pool(name="psd", bufs=4, space="PSUM"))
                po = ctxd.enter_context(tc.tile_pool(name="po", bufs=3))
                osb = None
                si = 0
                gs = 0  # store-group start tile
                for ct in range(NCT):
                    n = min(CTD, VCH - ct * CTD)
                    g = ct - gs  # position within the store group
                    if g == 0:
                        osb = po.tile([128, GRP, 512], I8, tag="osb")
                    for b in range(B):
                        pd = psd.tile([128, CTD, 256], F32, tag="pd",
                                      space="PSUM")
                        if ct == 0 and b == 0:
                            warm(pd[:, 0, :], 20)
                        for j in range(n):
                            v = ct * CTD + j
                            for kk in range(2):
                                nc.tensor.matmul(
                                    pd[:, j, :],
                                    wd_sb[:, 2 * kk:2 * kk + 2,
                                          v * 128:(v + 1) * 128],
                                    sT[b][:, 2 * kk:2 * kk + 2,
                                          HALO:HALO + SC],
                                    start=(kk == 0), stop=(kk == 1),
                                    perf_mode=DR)
                        dst = osb[:, g * CTD:g * CTD + n, b * SC:(b + 1) * SC]
                        if sched[si] == "act":
                            nc.scalar.activation(dst, pd[:, 0:n, :], AF.Copy)
                        else:
                            nc.vector.tensor_copy(dst, pd[:, 0:n, :])
                        si += 1
                    ship = (g == 3 or ct == NCT - 1
                            or (ct >= NCT - 4 and g == 1))
                    if ship:
                        r0 = gs * CTD * 128
                        r1 = (ct * CTD + n) * 128
                        dst = t_y.ap()[r0:r1, :]
                        nc.gpsimd.dma_start(
                            dst.rearrange("(j p) t -> p j t", p=128),
                            osb[:, 0:(ct - gs) * CTD + n, :])
                        gs = ct + 1

    nc.compile()
    return nc


def _get_module(variant="all"):
    key = f"nc:{variant}"
    if key not in _CACHE:
        _CACHE[key] = _build_module()
    return _CACHE[key]


def _prep_inputs(x, emb, w_mass, b_mass, w_f1, b_f1, w_f2, b_f2, cfl_raw,
                 w_dec, b_dec):
    x = np.asarray(x)
    emb = np.asarray(emb, dtype=np.float64)
    w_mass = np.asarray(w_mass, dtype=np.float64)
    b_mass = np.asarray(b_mass, dtype=np.float64)
    w_f1 = np.asarray(w_f1, dtype=np.float64)
    b_f1 = np.asarray(b_f1, dtype=np.float64)
    w_f2 = np.asarray(w_f2, dtype=np.float64)
    b_f2 = np.asarray(b_f2, dtype=np.float64)
    w_dec = np.asarray(w_dec, dtype=np.float32)
    dt = float(1.0 / (1.0 + np.exp(-np.float64(np.asarray(cfl_raw)))))

    bf16 = ml_dtypes.bfloat16
    fp8 = ml_dtypes.float8_e4m3

    # host-side embedding gather + transpose + fp8 quantization (pure data
    # movement; also shrinks per-core input traffic vs shipping all of emb)
    emb8 = (np.asarray(emb, np.float64) * SE).astype(bf16).astype(fp8)
    wm_in = np.ascontiguousarray(
        (w_mass * SW).reshape(DC, 128, D).transpose(1, 0, 2)
        .reshape(128, DC * D).astype(fp8))
    wf1_in = np.ascontiguousarray(
        (w_f1 * SW).reshape(2, DC, 128, D).transpose(2, 0, 1, 3)
        .reshape(128, 2 * DC * D).astype(fp8))
    wf2_in = np.ascontiguousarray(
        (w_f2 * SW).reshape(DC, 128, D).transpose(1, 0, 2)
        .reshape(128, DC * D).astype(fp8))
    wd_in = np.ascontiguousarray(
        (np.asarray(w_dec, np.float64) * SWD).reshape(DC, 128, V)
        .transpose(1, 0, 2).reshape(128, DC * V).astype(fp8))

    # host-exact bias folds (f64)
    b1p = b_f1 + C0 * w_f1.sum(0)                  # [D]
    hbar = np.tanh(b1p)                            # [D]
    b2p = b_f2 + hbar @ w_f2                       # [D]
    rowm = np.ascontiguousarray((2048.0 * b_mass)[None, :].astype(np.float32))
    row1 = np.ascontiguousarray((2048.0 * b1p)[None, :].astype(np.float32))
    row2 = np.ascontiguousarray((2048.0 * b2p)[None, :].astype(np.float32))
    ones_in = np.ones((1, WFP), np.float32)
    hb_in = np.ascontiguousarray(
        (SH * hbar).reshape(DC, 128).T.astype(np.float32))
    c32_in = np.full((128, DC), C32, np.float32)
    s32_in = np.full((128, 1), 32.0, np.float32)
    dt_in = np.full((128, 1), 32.0 * dt, dtype=np.float32)

    in_maps = []
    for c in range(NCORES):
        sedge = c * SC - HALO
        idx = np.zeros(GT * 128, dtype=np.int32)
        for b in range(B):
            t = np.arange(WM)
            sc = np.clip(sedge + t, 0, S - 1)
            idx[b * WM:(b + 1) * WM] = x[b, sc]
        # eT[p, g, slot] = emb8[idx[slot], g*128 + p]
        eg = emb8[idx]                                # [GT*128, D]
        eT_in = np.ascontiguousarray(
            eg.T.reshape(DC, 128, GT * 128).transpose(1, 0, 2)
            .reshape(128, DC * GT * 128))

        j = np.arange(WFP)
        gp = sedge + j
        fm = ((gp >= 0) & (gp <= S - 2)).astype(np.float32)
        mskl = np.ascontiguousarray(
            np.broadcast_to(fm[0:HALO], (128, HALO)).astype(bf16))
        mskr = np.ascontiguousarray(
            np.broadcast_to(fm[WF - HALO:WF], (128, HALO)).astype(bf16))

        in_maps.append({
            "t_eT": eT_in, "t_mskl": mskl, "t_mskr": mskr, "t_dt": dt_in,
            "t_s32": s32_in, "t_c32": c32_in, "t_hb": hb_in,
            "t_wm": wm_in, "t_wf1": wf1_in, "t_wf2": wf2_in,
            "t_rowm": rowm, "t_row1": row1, "t_row2": row2,
            "t_ones": ones_in, "t_wd": wd_in,
        })
    return in_maps


def _edge_exact(inputs, y, edge=8, pad=4):
    """Exact f64 reference for the first/last `edge` sequence positions.
    The device drops the clip-at-EPS and quantizes the large boundary
    deltas in fp8; both effects are confined to cells within K=3 of the
    global edges, so recompute those tokens' logits on the host."""
    x = np.asarray(inputs["x"])
    emb = np.asarray(inputs["emb"], np.float64)
    w_mass = np.asarray(inputs["w_mass"], np.float64)
    b_mass = np.asarray(inputs["b_mass"], np.float64)
    w_f1 = np.asarray(inputs["w_f1"], np.float64)
    b_f1 = np.asarray(inputs["b_f1"], np.float64)
    w_f2 = np.asarray(inputs["w_f2"], np.float64)
    b_f2 = np.asarray(inputs["b_f2"], np.float64)
    w_dec = np.asarray(inputs["w_dec"], np.float64)
    b_dec = np.asarray(inputs["b_dec"], np.float64)
    dt = 1.0 / (1.0 + np.exp(-np.float64(np.asarray(inputs["cfl_raw"]))))

    def sp(z):
        return np.logaddexp(0.0, z)

    W = edge + pad
    for side in (0, 1):
        pos = np.arange(0, W) if side == 0 else np.arange(S - W, S)
        m = sp(emb[x[:, pos]] @ w_mass + b_mass) + EPS
        for _ in range(KSTEPS):
            lr = np.concatenate([m[:, :-1], m[:, 1:]], axis=-1)
            F = sp(np.tanh(lr @ w_f1 + b_f1) @ w_f2 + b_f2)
            zpad = np.zeros((B, 1, D))
            if side == 0:
                Fl = np.concatenate([zpad, F], axis=1)       # global edge
                Fr = np.concatenate([F, F[:, -1:]], axis=1)  # window edge
            else:
                Fl = np.concatenate([F[:, 0:1], F], axis=1)
                Fr = np.concatenate([F, zpad], axis=1)
            m = np.clip(m + dt * (Fl - Fr), EPS, None)
        logits = m @ w_dec + b_dec
        if side == 0:
            y[:, 0:edge, :] = logits[:, 0:edge, :].astype(np.float32)
        else:
            y[:, S - edge:S, :] = logits[:, pad:, :].astype(np.float32)


def kernel(**inputs) -> np.ndarray:
    global LAST_RESULTS
    import os
    nc = _get_module()
    in_maps = _prep_inputs(**inputs)
    try:
        res = run_bass_kernel_spmd(nc, in_maps, core_ids=list(range(NCORES)))
    except (ImportError, ModuleNotFoundError):
        if os.environ.get("BASS_TRACE"):
            os.environ["BASS_NEVER_TRACE"] = "1"
            res = run_bass_kernel_spmd(nc, in_maps,
                                       core_ids=list(range(NCORES)))
        else:
            raise
    LAST_RESULTS = res

    w_dec = np.asarray(inputs["w_dec"], dtype=np.float32)
    b_dec = np.asarray(inputs["b_dec"], dtype=np.float32)
    L0 = (C0 * w_dec.sum(0) + b_dec).astype(np.float32)  # [V]
    wmax2 = float(np.linalg.norm(w_dec, axis=0).max())
    lim = (127.0 / KOUT) / 1.08

    y = np.empty((B, S, V), dtype=np.float32)
    for c in range(NCORES):
        blk = res.results[c]["t_y"].astype(np.float32) / KOUT  # [V, B*SC]
        blk = blk.reshape(V, B, SC).transpose(1, 2, 0)         # [B, SC, V]
        y[:, c * SC:(c + 1) * SC, :] = blk + L0[None, None, :]
        # final state: [128, B, DC, WMA]; delta[t, d] = s[p, b, g, HALO+t]/32
        ss = res.results[c]["t_s"].astype(np.float32).reshape(
            128, B, DC, WMA)
        for b in range(B):
            sd = ss[:, b, :, HALO:HALO + SC]            # [128, DC, SC]
            delta = sd.transpose(2, 1, 0).reshape(SC, D) / SDELT
            bt = np.linalg.norm(delta, axis=1)
            for t in np.nonzero(bt * wmax2 > lim)[0]:
                s_pos = c * SC + t
                y[b, s_pos, :] = (delta[t] @ w_dec) + L0
    _edge_exact(inputs, y)
    return y


# revision 18
# speedup vs baseline: 1.0934x; 1.0018x over previous
"""Trainium2 Bass kernel for nn_DTFN (mass/flux stencil + vocab decoder).

Strategy (8 NeuronCores, SPMD single NEFF, token-parallel, NO collectives):
  - Sequence-parallel mass+flux: each core computes its S/8=256 positions per
    batch with a K=3 halo; global zero-flux boundaries via per-core edge
    masks that zero the edge flux cells.
  - The flux state lives ONLY as fp8 (e4m3) in mean-shifted units:
      s = 32*(m - C0),  C0 = 0.6875 (exact in e4m3).
    |s| <= ~2 for typical tokens so fp8 resolution on m matches bf16. The
    update s' = max(s + 32*dt*(F[i-1]-F[i]), -32*(C0-EPS)) keeps the
    reference clip-at-EPS semantics exactly.
  - ALL matmuls are fp8 DoubleRow (0.5 cyc/col): mass (emb x w_mass), flux
    h = tanh(lr@w1+b1) and F = softplus(h@w2+b2), and the vocab decode.
    Accuracy is preserved by mean shifts with the rank-1 remainders folded
    into biases on the host (exact, f64):
      lr ~= C0*ones + delta  -> b1' = b1 + C0*colsum(w1)
      h  ~= hbar + dh, hbar = tanh(b1') -> b2' = b2 + hbar@w2
    Biases enter the psum through one f32r rank-1 matmul per 128-chunk
    (bias_row (x) ones), so the scalar-engine activations can run BATCHED
    over all 4 d-chunks (bias of an activation op is per-partition only).
  - Activation tables: softplus = Ln(1+Exp(.)). Exp+Ln share the
    natural_log_exp table; Tanh needs exp_and_others, so each flux step pays
    exactly 2 table swaps (explicitly placed).
  - Token-parallel decoder: each core decodes its OWN 512 tokens against the
    FULL vocab (streams all of w_dec in fp8, no collectives). psum -> int8
    converts (the true wall: ~131k elem/partition across Act+DVE) run as
    [128, 8 vocab-chunks, 256 tok] tiles split greedily across Act and DVE.
  - Output int8 in units of 1/256 logits; host adds the exact rank-1 term
    C0*colsum(w_dec)+b_dec and recomputes the few saturated (boundary)
    tokens from the stored final state s.
"""

import numpy as np
import ml_dtypes
from contextlib import ExitStack

import concourse.bass as bass
import concourse.bacc as bacc
import concourse.mybir as mybir
import concourse.tile as tile
from concourse.bass_utils import run_bass_kernel_spmd
from concourse.bass import _add_dep_helper

F32 = mybir.dt.float32
F32R = mybir.dt.float32r
BF16 = mybir.dt.bfloat16
FP8 = mybir.dt.float8e4
I8 = mybir.dt.int8
I32 = mybir.dt.int32
AF = mybir.ActivationFunctionType
DR = mybir.MatmulPerfMode.DoubleRow
ALU = mybir.AluOpType

V, D, KSTEPS, B, S = 32000, 512, 3, 2, 2048
EPS = 1e-6
NCORES = 8
SC = S // NCORES          # 256 seq positions per batch per core
HALO = KSTEPS             # 3
WM = SC + 2 * HALO        # 262 m-cells per batch per core
WMA = WM + 2              # 264 allocated (pad cells read as 0)
WF = WM - 1               # 261 real flux pairs
WFP = WF + 1              # 262 computed pairs
NTOK = B * WM             # 524 gathered tokens per core
GT = (NTOK + 127) // 128  # 5 gather tiles
DC = D // 128             # 4 d-chunks
C0 = 0.6875               # mean shift, exactly representable in e4m3
C32 = 32.0 * (C0 - EPS)   # state offset: s = 32*mt - C32, mt = m - EPS
SE = 32.0                 # emb scale (host, folded into bf16 emb)
SW = 64.0                 # flux weight scale
SDELT = 32.0              # state scale
SH = 32.0                 # dh scale
SWD = 8.0                 # w_dec scale
KOUT = SDELT * SWD        # 256: psum/int8 units per logit
PSC = 1.0 / 2048.0        # psum -> pre-activation scale (32*64)
VCH = V // 128            # 250 vocab chunks of 128
NWCH = 10                 # w_dec streamed in 10 column chunks
WCOLS = V // NWCH         # 6400 columns per streamed chunk
VPC = WCOLS // 128        # 50 vocab chunks per streamed chunk
CTD = 4                   # vocab chunks per decode psum tile (1 batch)
NCT = (VCH + CTD - 1) // CTD  # 32 decode tiles per batch
GRP = 16                  # vocab chunks per store DMA (both batches)

_CACHE: dict = {}
LAST_RESULTS = None


def _conv_schedule():
    """Greedy least-finish-time assignment of the 2*NCT convert tiles to
    Act/DVE. Costs from the TimelineSim model for [128, n*256] f32->int8."""
    out = []
    load = {"act": 0.0, "dve": 0.0}
    for ct in range(NCT):
        n = min(CTD, VCH - ct * CTD)
        cost = {"act": n * 256 * 0.8333 + 185.0, "dve": n * 256 * 1.0417 + 250.0}
        for _b in range(B):
            e = min(cost, key=lambda k: load[k] + cost[k])
            load[e] += cost[e]
            out.append(e)
    return out


def _build_module():
    nc = bacc.Bacc("TRN2", target_bir_lowering=False, debug=False,
                   num_devices=NCORES)
    from concourse.hw_specs import get_activation_tables
    tables = list(get_activation_tables(nc.m.arch))
    NLE_SET = tables.index("natural_log_exp_and_others")
    E_SET = tables.index("exp_and_others")

    # --- per-core DRAM I/O ---
    t_mskl = nc.dram_tensor("t_mskl", [128, HALO], BF16, kind="ExternalInput")
    t_mskr = nc.dram_tensor("t_mskr", [128, HALO], BF16, kind="ExternalInput")
    t_dt = nc.dram_tensor("t_dt", [128, 1], F32, kind="ExternalInput")
    t_s32 = nc.dram_tensor("t_s32", [128, 1], F32, kind="ExternalInput")
    t_c32 = nc.dram_tensor("t_c32", [128, DC], F32, kind="ExternalInput")
    t_hb = nc.dram_tensor("t_hb", [128, DC], F32, kind="ExternalInput")
    t_wm = nc.dram_tensor("t_wm", [128, DC * D], FP8, kind="ExternalInput")
    t_wf1 = nc.dram_tensor("t_wf1", [128, 2 * DC * D], FP8, kind="ExternalInput")
    t_wf2 = nc.dram_tensor("t_wf2", [128, DC * D], FP8, kind="ExternalInput")
    t_rowm = nc.dram_tensor("t_rowm", [1, D], F32R, kind="ExternalInput")
    t_row1 = nc.dram_tensor("t_row1", [1, D], F32R, kind="ExternalInput")
    t_row2 = nc.dram_tensor("t_row2", [1, D], F32R, kind="ExternalInput")
    t_ones = nc.dram_tensor("t_ones", [1, WFP], F32R, kind="ExternalInput")
    t_eT = nc.dram_tensor("t_eT", [128, DC * GT * 128], FP8, kind="ExternalInput")
    t_wd = nc.dram_tensor("t_wd", [128, DC * V], FP8, kind="ExternalInput")
    t_y = nc.dram_tensor("t_y", [VCH * 128, B * SC], I8, kind="ExternalOutput")
    # final state s = 32*(m - C0), so the host can detect and recompute the
    # saturated (global-boundary) tokens
    t_s = nc.dram_tensor("t_s", [128, B * DC * WMA], FP8, kind="ExternalOutput")

    with tile.TileContext(nc) as tc:
        with ExitStack() as ctx:
            pw = ctx.enter_context(tc.tile_pool(name="pw", bufs=1))
            pwd = ctx.enter_context(tc.tile_pool(name="pwd", bufs=1))
            pfl = ctx.enter_context(tc.tile_pool(name="pfl", bufs=1))

            # ---- small loads ----
            eT = pw.tile([128, DC, GT * 128], FP8)
            nc.sync.dma_start(
                eT[:], t_eT.ap().rearrange("p (g t) -> p g t", g=DC))
            wm_sb = pw.tile([128, DC, D], FP8)
            nc.sync.dma_start(wm_sb[:], t_wm.ap().rearrange("p (g d) -> p g d", g=DC))
            rowm = pw.tile([1, D], F32R)
            nc.sync.dma_start(rowm[:], t_rowm.ap())
            row1 = pw.tile([1, D], F32R)
            nc.sync.dma_start(row1[:], t_row1.ap())
            row2 = pw.tile([1, D], F32R)
            nc.sync.dma_start(row2[:], t_row2.ap())
            ones = pw.tile([1, WFP], F32R)
            nc.sync.dma_start(ones[:], t_ones.ap())
            mskl_sb = pw.tile([128, HALO], BF16)
            nc.sync.dma_start(mskl_sb[:], t_mskl.ap())
            mskr_sb = pw.tile([128, HALO], BF16)
            nc.sync.dma_start(mskr_sb[:], t_mskr.ap())
            dt_sb = pw.tile([128, 1], F32)
            nc.sync.dma_start(dt_sb[:], t_dt.ap())
            s32_sb = pw.tile([128, 1], F32)
            nc.sync.dma_start(s32_sb[:], t_s32.ap())
            c32_sb = pw.tile([128, DC], F32)
            nc.sync.dma_start(c32_sb[:], t_c32.ap())
            hb_sb = pw.tile([128, DC], F32)
            nc.sync.dma_start(hb_sb[:], t_hb.ap())
            wf1_sb = pw.tile([128, 2, DC, D], FP8)
            wf2_sb = pw.tile([128, DC, D], FP8)

            # persistent per-batch state s (fp8) and work tiles
            sT = [pfl.tile([128, DC, WMA], FP8, tag=f"sT{b}", name=f"sT{b}") for b in range(B)]
            hT = [pfl.tile([128, DC, WFP], BF16, tag=f"hT{b}", name=f"hT{b}") for b in range(B)]
            dh8 = [pfl.tile([128, DC, WFP], FP8, tag=f"dh{b}", name=f"dh{b}") for b in range(B)]
            ftmp = [pfl.tile([128, DC, WFP], BF16, tag=f"ft{b}", name=f"ft{b}") for b in range(B)]
            F_sb = [pfl.tile([128, DC, WFP], BF16, tag=f"F{b}", name=f"F{b}") for b in range(B)]
            Fd = [pfl.tile([128, DC, WF - 1], BF16, tag=f"Fd{b}", name=f"Fd{b}") for b in range(B)]
            mt0 = [pfl.tile([128, DC, WM], BF16, tag=f"m0{b}", name=f"m0{b}") for b in range(B)]
            for b in range(B):
                nc.vector.memset(sT[b][:, :, WM:WMA], 0.0)  # pad cells

            # flux weights stream before the wdec chunks
            wf1_i = nc.sync.dma_start(
                wf1_sb[:],
                t_wf1.ap().rearrange("p (s g d) -> p s g d", s=2, g=DC))
            wf2_i = nc.sync.dma_start(
                wf2_sb[:],
                t_wf2.ap().rearrange("p (g d) -> p g d", g=DC))

            # decoder weights: fully resident in SBUF, streamed in slices
            # behind the flux weights so the decode window is free of wdec
            # DMA traffic (stores get the whole DMA device)
            wd_sb = pwd.tile([128, DC, V], FP8, tag="wd")
            t_wd_v = t_wd.ap().rearrange("p (g v) -> p g v", g=DC)
            for c in range(NWCH):
                wi = nc.sync.dma_start(
                    wd_sb[:, :, c * WCOLS:(c + 1) * WCOLS],
                    t_wd_v[:, :, c * WCOLS:(c + 1) * WCOLS])
                if c == 0:
                    _add_dep_helper(wi.ins, wf2_i.ins, sync=True,
                                    reason="wdec stream after flux weights")

            # ---- explicit activation-table management ----
            def load_table(set_id, after=None):
                ld = mybir.InstLoadActFuncSet(
                    name=nc.get_next_instruction_name(), ins=[], outs=[],
                    act_func_set_id=set_id)
                bi = nc.scalar.add_instruction(ld)
                if after is not None:
                    _add_dep_helper(bi.ins, after.ins, sync=True,
                                    reason="table load order")
                return bi

            last_act = [None]

            def anchored_load(set_id):
                ld = load_table(set_id, after=last_act[0])
                last_act[0] = ld
                return ld

            ld_nle = load_table(NLE_SET)
            last_act[0] = ld_nle
            last_mm = [None]

            def warm(dst, n, anchor=None):
                """n tiny chained matmuls into `dst` (a psum slice that the
                next real start=True matmul overwrites) keep the PE p-state
                ramp alive through an engine-chain gap. `anchor` delays the
                chain start so it does not run (and finish) too early."""
                prev = anchor if anchor is not None else last_mm[0]
                for _ in range(n):
                    ji = nc.tensor.matmul(
                        dst[0:64, 0:64], wm_sb[:, 0, 0:64],
                        wm_sb[:, 0, 64:128],
                        start=True, stop=True, skip_group_check=True)
                    if prev is not None:
                        _add_dep_helper(ji.ins, prev.ins, sync=True,
                                        reason="pe warm chain")
                    prev = ji
                last_mm[0] = prev

            # ---- psum pools: A (batch 0), TR (transposes), B (batch 1) ----
            ctxA = ExitStack()
            psA = ctxA.enter_context(tc.tile_pool(name="psA", bufs=1, space="PSUM"))
            # ---- mass (per batch): s0 = 32*(softplus(e@wm + bm) - C8) ----
            first_exp = [None, None]
            last_ln = [None, None]
            last_tanh = [None, None]
            last_fd0 = [None]

            def emit_mass(b, pool):
                pm = pool.tile([128, DC, 512], F32, tag="fx", space="PSUM")
                if b == 0:
                    warm(pm[:, 0, :], 8)
                for q in range(DC):
                    for kk in range(2):
                        nc.tensor.matmul(
                            pm[:, q, 0:WM],
                            wm_sb[:, 2 * kk:2 * kk + 2, q * 128:(q + 1) * 128],
                            eT[:, 2 * kk:2 * kk + 2, b * WM:b * WM + WM],
                            start=(kk == 0), stop=False, perf_mode=DR)
                    bi = nc.tensor.matmul(
                        pm[:, q, 0:WM], rowm[:, q * 128:(q + 1) * 128],
                        ones[:, 0:WM], start=False, stop=True)
                    last_mm[0] = bi
                ei = nc.scalar.activation(ftmp[b][:, :, 0:WM], pm[:, :, 0:WM],
                                          AF.Exp, scale=PSC)
                first_exp[b] = ei
                li = nc.scalar.activation(mt0[b][:], ftmp[b][:, :, 0:WM],
                                          AF.Ln, bias=1.0)
                last_act[0] = li
                last_ln[b] = li
                # s0 = 32*mt - c32 (DVE, fp8 out)
                nc.vector.scalar_tensor_tensor(
                    out=sT[b][:, :, 0:WM], in0=mt0[b][:], scalar=s32_sb[:, 0:1],
                    in1=c32_sb[:, :, None].to_broadcast([128, DC, WM]),
                    op0=ALU.mult, op1=ALU.subtract)

            emit_mass(0, psA)
            _add_dep_helper(first_exp[0].ins, ld_nle.ins, sync=True,
                            reason="mass exp after nle load")
            ctxB = ExitStack()
            psB = ctxB.enter_context(tc.tile_pool(name="psB", bufs=1, space="PSUM"))
            emit_mass(1, psB)
            pools = [psA, psB]

            # ---- K flux steps, batches staggered by half a phase ----
            for k in range(KSTEPS):
                ld_e = anchored_load(E_SET)
                ph = [None, None]
                for b in range(B):
                    ph[b] = pools[b].tile([128, DC, 512], F32, tag="fx",
                                          name=f"ph{b}", space="PSUM")
                    if b == 0:
                        warm(ph[0][:, 0, :], 24,
                             anchor=last_fd0[0] if last_fd0[0] is not None
                             else last_ln[0])
                    for kk in range(2):
                        for q in range(DC):
                            for sh in range(2):
                                nc.tensor.matmul(
                                    ph[b][:, q, 0:WFP],
                                    wf1_sb[:, sh, 2 * kk:2 * kk + 2,
                                           q * 128:(q + 1) * 128],
                                    sT[b][:, 2 * kk:2 * kk + 2, sh:sh + WFP],
                                    start=(sh == 0 and kk == 0), stop=False,
                                    perf_mode=DR)
                    for q in range(DC):
                        bi = nc.tensor.matmul(
                            ph[b][:, q, 0:WFP], row1[:, q * 128:(q + 1) * 128],
                            ones[:, 0:WFP], start=False, stop=True)
                        last_mm[0] = bi
                for b in range(B):
                    ti = nc.scalar.activation(hT[b][:], ph[b][:, :, 0:WFP],
                                              AF.Tanh, scale=PSC)
                    if b == 0:
                        _add_dep_helper(ti.ins, ld_e.ins, sync=True,
                                        reason="tanh after E load")
                    last_act[0] = ti
                    last_tanh[b] = ti
                ld_n = anchored_load(NLE_SET)
                for b in range(B):
                    # dh = 32*h - 32*hbar (fp8)
                    nc.vector.scalar_tensor_tensor(
                        out=dh8[b][:], in0=hT[b][:], scalar=s32_sb[:, 0:1],
                        in1=hb_sb[:, :, None].to_broadcast([128, DC, WFP]),
                        op0=ALU.mult, op1=ALU.subtract)
                pf = [None, None]
                for b in range(B):
                    pf[b] = pools[b].tile([128, DC, 512], F32, tag="fx",
                                          name=f"pf{b}", space="PSUM")
                    if b == 0:
                        warm(pf[0][:, 0, :], 26, anchor=last_tanh[0])
                    for q in range(DC):
                        for kk in range(2):
                            nc.tensor.matmul(
                                pf[b][:, q, 0:WFP],
                                wf2_sb[:, 2 * kk:2 * kk + 2,
                                       q * 128:(q + 1) * 128],
                                dh8[b][:, 2 * kk:2 * kk + 2, 0:WFP],
                                start=(kk == 0), stop=False, perf_mode=DR)
                        bi = nc.tensor.matmul(
                            pf[b][:, q, 0:WFP], row2[:, q * 128:(q + 1) * 128],
                            ones[:, 0:WFP], start=False, stop=True)
                        last_mm[0] = bi
                # exp runs in the E table (exp_and_others has Exp); the NLE
                # load slots between the exps and the lns
                for b in range(B):
                    ei = nc.scalar.activation(ftmp[b][:], pf[b][:, :, 0:WFP],
                                              AF.Exp, scale=PSC)
                    if b == 0:
                        _add_dep_helper(ei.ins, ld_n.ins, sync=True,
                                        reason="exp after nle load")
                    last_act[0] = ei
                    nc.gpsimd.tensor_tensor(
                        out=ftmp[b][:, :, 0:HALO], in0=ftmp[b][:, :, 0:HALO],
                        in1=mskl_sb[:, None, :].to_broadcast([128, DC, HALO]),
                        op=ALU.mult)
                    nc.gpsimd.tensor_tensor(
                        out=ftmp[b][:, :, WF - HALO:WF],
                        in0=ftmp[b][:, :, WF - HALO:WF],
                        in1=mskr_sb[:, None, :].to_broadcast([128, DC, HALO]),
                        op=ALU.mult)
                    li = nc.scalar.activation(F_sb[b][:], ftmp[b][:], AF.Ln,
                                              bias=1.0)
                    last_act[0] = li
                    last_ln[b] = li
                # s' = 32*dt*(F[i-1]-F[i]) + s.  The clip-at-EPS max is
                # dropped: it only binds at the global boundary cells, which
                # the host recomputes exactly.  Batch 0 (the chain batch)
                # updates in chunk-pair halves so the next h matmuls (kk
                # outer) can start after the first half.
                for b in range(B):
                    halves = [(0, 2), (2, 4)] if b == 0 else [(0, 4)]
                    for (q0, q1) in halves:
                        qn = q1 - q0
                        fi = nc.vector.tensor_tensor(
                            out=Fd[b][:, q0:q1, :],
                            in0=F_sb[b][:, q0:q1, 0:WF - 1],
                            in1=F_sb[b][:, q0:q1, 1:WF], op=ALU.subtract)
                        if b == 0 and q0 == 0:
                            last_fd0[0] = fi
                        nc.vector.scalar_tensor_tensor(
                            out=sT[b][:, q0:q1, 1:WM - 1],
                            in0=Fd[b][:, q0:q1, :],
                            scalar=dt_sb[:, 0:1],
                            in1=sT[b][:, q0:q1, 1:WM - 1],
                            op0=ALU.mult, op1=ALU.add)

            # final state out (host overflow detection + recompute)
            t_s_v = t_s.ap().rearrange("p (b g t) -> p b g t", b=B, g=DC)
            for b in range(B):
                nc.sync.dma_start(t_s_v[:, b, :, :], sT[b][:])

            ctxB.close()
            ctxA.close()

            # ---- decode: fp8 DR matmuls + psum->int8 converts + stores ----
            sched = _conv_schedule()
            with ExitStack() as ctxd:
                psd = ctxd.enter_context(
                    tc.tile_pool(name="psd", bufs=4, space="PSUM"))
                po = ctxd.enter_context(tc.tile_pool(name="po", bufs=3))
                osb = None
                si = 0
                gs = 0  # store-group start tile
                for ct in range(NCT):
                    n = min(CTD, VCH - ct * CTD)
                    g = ct - gs  # position within the store group
                    if g == 0:
                        osb = po.tile([128, GRP, 512], I8, tag="osb")
                    for b in range(B):
                        pd = psd.tile([128, CTD, 256], F32, tag="pd",
                                      space="PSUM")
                        if ct == 0 and b == 0:
                            warm(pd[:, 0, :], 20)
                        for j in range(n):
                            v = ct * CTD + j
                            for kk in range(2):
                                nc.tensor.matmul(
                                    pd[:, j, :],
                                    wd_sb[:, 2 * kk:2 * kk + 2,
                                          v * 128:(v + 1) * 128],
                                    sT[b][:, 2 * kk:2 * kk + 2,
                                          HALO:HALO + SC],
                                    start=(kk == 0), stop=(kk == 1),
                                    perf_mode=DR)
                        dst = osb[:, g * CTD:g * CTD + n, b * SC:(b + 1) * SC]
                        if sched[si] == "act":
                            nc.scalar.activation(dst, pd[:, 0:n, :], AF.Copy)
                        else:
                            nc.vector.tensor_copy(dst, pd[:, 0:n, :])
                        si += 1
                    ship = (g == 3 or ct == NCT - 1
                            or (ct >= NCT - 4 and g == 1))
                    if ship:
                        r0 = gs * CTD * 128
                        r1 = (ct * CTD + n) * 128
                        dst = t_y.ap()[r0:r1, :]
                        nc.gpsimd.dma_start(
                            dst.rearrange("(j p) t -> p j t", p=128),
                            osb[:, 0:(ct - gs) * CTD + n, :])
                        gs = ct + 1

    nc.compile()
    return nc


def _get_module(variant="all"):
    key = f"nc:{variant}"
    if key not in _CACHE:
        _CACHE[key] = _build_module()
    return _CACHE[key]


def _prep_inputs(x, emb, w_mass, b_mass, w_f1, b_f1, w_f2, b_f2, cfl_raw,
                 w_dec, b_dec):
    x = np.asarray(x)
    emb = np.asarray(emb, dtype=np.float64)
    w_mass = np.asarray(w_mass, dtype=np.float64)
    b_mass = np.asarray(b_mass, dtype=np.float64)
    w_f1 = np.asarray(w_f1, dtype=np.float64)
    b_f1 = np.asarray(b_f1, dtype=np.float64)
    w_f2 = np.asarray(w_f2, dtype=np.float64)
    b_f2 = np.asarray(b_f2, dtype=np.float64)
    w_dec = np.asarray(w_dec, dtype=np.float32)
    dt = float(1.0 / (1.0 + np.exp(-np.float64(np.asarray(cfl_raw)))))

    bf16 = ml_dtypes.bfloat16
    fp8 = ml_dtypes.float8_e4m3

    # host-side embedding gather + transpose + fp8 quantization (pure data
    # movement; also shrinks per-core input traffic vs shipping all of emb)
    emb8 = (np.asarray(emb, np.float64) * SE).astype(bf16).astype(fp8)
    wm_in = np.ascontiguousarray(
        (w_mass * SW).reshape(DC, 128, D).transpose(1, 0, 2)
        .reshape(128, DC * D).astype(fp8))
    wf1_in = np.ascontiguousarray(
        (w_f1 * SW).reshape(2, DC, 128, D).transpose(2, 0, 1, 3)
        .reshape(128, 2 * DC * D).astype(fp8))
    wf2_in = np.ascontiguousarray(
        (w_f2 * SW).reshape(DC, 128, D).transpose(1, 0, 2)
        .reshape(128, DC * D).astype(fp8))
    wd_in = np.ascontiguousarray(
        (np.asarray(w_dec, np.float64) * SWD).reshape(DC, 128, V)
        .transpose(1, 0, 2).reshape(128, DC * V).astype(fp8))

    # host-exact bias folds (f64)
    b1p = b_f1 + C0 * w_f1.sum(0)                  # [D]
    hbar = np.tanh(b1p)                            # [D]
    b2p = b_f2 + hbar @ w_f2                       # [D]
    rowm = np.ascontiguousarray((2048.0 * b_mass)[None, :].astype(np.float32))
    row1 = np.ascontiguousarray((2048.0 * b1p)[None, :].astype(np.float32))
    row2 = np.ascontiguousarray((2048.0 * b2p)[None, :].astype(np.float32))
    ones_in = np.ones((1, WFP), np.float32)
    hb_in = np.ascontiguousarray(
        (SH * hbar).reshape(DC, 128).T.astype(np.float32))
    c32_in = np.full((128, DC), C32, np.float32)
    s32_in = np.full((128, 1), 32.0, np.float32)
    dt_in = np.full((128, 1), 32.0 * dt, dtype=np.float32)

    in_maps = []
    for c in range(NCORES):
        sedge = c * SC - HALO
        idx = np.zeros(GT * 128, dtype=np.int32)
        for b in range(B):
            t = np.arange(WM)
            sc = np.clip(sedge + t, 0, S - 1)
            idx[b * WM:(b + 1) * WM] = x[b, sc]
        # eT[p, g, slot] = emb8[idx[slot], g*128 + p]
        eg = emb8[idx]                                # [GT*128, D]
        eT_in = np.ascontiguousarray(
            eg.T.reshape(DC, 128, GT * 128).transpose(1, 0, 2)
            .reshape(128, DC * GT * 128))

        j = np.arange(WFP)
        gp = sedge + j
        fm = ((gp >= 0) & (gp <= S - 2)).astype(np.float32)
        mskl = np.ascontiguousarray(
            np.broadcast_to(fm[0:HALO], (128, HALO)).astype(bf16))
        mskr = np.ascontiguousarray(
            np.broadcast_to(fm[WF - HALO:WF], (128, HALO)).astype(bf16))

        in_maps.append({
            "t_eT": eT_in, "t_mskl": mskl, "t_mskr": mskr, "t_dt": dt_in,
            "t_s32": s32_in, "t_c32": c32_in, "t_hb": hb_in,
            "t_wm": wm_in, "t_wf1": wf1_in, "t_wf2": wf2_in,
            "t_rowm": rowm, "t_row1": row1, "t_row2": row2,
            "t_ones": ones_in, "t_wd": wd_in,
        })
    return in_maps


def _edge_exact(inputs, y, edge=8, pad=4):
    """Exact f64 reference for the first/last `edge` sequence positions.
    The device drops the clip-at-EPS and quantizes the large boundary
    deltas in fp8; both effects are confined to cells within K=3 of the
    global edges, so recompute those tokens' logits on the host."""
    x = np.asarray(inputs["x"])
    emb = np.asarray(inputs["emb"], np.float64)
    w_mass = np.asarray(inputs["w_mass"], np.float64)
    b_mass = np.asarray(inputs["b_mass"], np.float64)
    w_f1 = np.asarray(inputs["w_f1"], np.float64)
    b_f1 = np.asarray(inputs["b_f1"], np.float64)
    w_f2 = np.asarray(inputs["w_f2"], np.float64)
    b_f2 = np.asarray(inputs["b_f2"], np.float64)
    w_dec = np.asarray(inputs["w_dec"], np.float64)
    b_dec = np.asarray(inputs["b_dec"], np.float64)
    dt = 1.0 / (1.0 + np.exp(-np.float64(np.asarray(inputs["cfl_raw"]))))

    def sp(z):
        return np.logaddexp(0.0, z)

    W = edge + pad
    for side in (0, 1):
        pos = np.arange(0, W) if side == 0 else np.arange(S - W, S)
        m = sp(emb[x[:, pos]] @ w_mass + b_mass) + EPS
        for _ in range(KSTEPS):
            lr = np.concatenate([m[:, :-1], m[:, 1:]], axis=-1)
            F = sp(np.tanh(lr @ w_f1 + b_f1) @ w_f2 + b_f2)
            zpad = np.zeros((B, 1, D))
            if side == 0:
                Fl = np.concatenate([zpad, F], axis=1)       # global edge
                Fr = np.concatenate([F, F[:, -1:]], axis=1)  # window edge
            else:
                Fl = np.concatenate([F[:, 0:1], F], axis=1)
                Fr = np.concatenate([F, zpad], axis=1)
            m = np.clip(m + dt * (Fl - Fr), EPS, None)
        logits = m @ w_dec + b_dec
        if side == 0:
            y[:, 0:edge, :] = logits[:, 0:edge, :].astype(np.float32)
        else:
            y[:, S - edge:S, :] = logits[:, pad:, :].astype(np.float32)


def kernel(**inputs) -> np.ndarray:
    global LAST_RESULTS
    import os
    nc = _get_module()
    in_maps = _prep_inputs(**inputs)
    try:
        res = run_bass_kernel_spmd(nc, in_maps, core_ids=list(range(NCORES)))
    except (ImportError, ModuleNotFoundError):
        if os.environ.get("BASS_TRACE"):
            os.environ["BASS_NEVER_TRACE"] = "1"
            res = run_bass_kernel_spmd(nc, in_maps,
                                       core_ids=list(range(NCORES)))
        else:
            raise
    LAST_RESULTS = res

    w_dec = np.asarray(inputs["w_dec"], dtype=np.float32)
    b_dec = np.asarray(inputs["b_dec"], dtype=np.float32)
    L0 = (C0 * w_dec.sum(0) + b_dec).astype(np.float32)  # [V]
    wmax2 = float(np.linalg.norm(w_dec, axis=0).max())
    lim = (127.0 / KOUT) / 1.08

    y = np.empty((B, S, V), dtype=np.float32)
    for c in range(NCORES):
        blk = res.results[c]["t_y"].astype(np.float32) / KOUT  # [V, B*SC]
        blk = blk.reshape(V, B, SC).transpose(1, 2, 0)         # [B, SC, V]
        y[:, c * SC:(c + 1) * SC, :] = blk + L0[None, None, :]
        # final state: [128, B, DC, WMA]; delta[t, d] = s[p, b, g, HALO+t]/32
        ss = res.results[c]["t_s"].astype(np.float32).reshape(
            128, B, DC, WMA)
        for b in range(B):
            sd = ss[:, b, :, HALO:HALO + SC]            # [128, DC, SC]
            delta = sd.transpose(2, 1, 0).reshape(SC, D) / SDELT
            bt = np.linalg.norm(delta, axis=1)
            for t in np.nonzero(bt * wmax2 > lim)[0]:
                s_pos = c * SC + t
                y[b, s_pos, :] = (delta[t] @ w_dec) + L0
    _edge_exact(inputs, y)
    return y
